# revision 12
# baseline (speedup 1.0000x reference)
"""Distributed VQE kernel for 8 TRN2 NeuronCores.

Math: each part's circuit is
    |psi> = P . U2 . P . U1 |0>
where U1/U2 are tensor products of single-qubit RZ*RY gates and P is the
CNOT-ladder basis permutation |x> -> |prefix_xor(x)>.  U1|0> is a product
state; relabeling through P keeps a Markov (bond-2) structure, applying U2
keeps bond 2, and the final P relabeling lifts it to bond 4.  So the full
2^20-amplitude state is an MPS of bond dimension 4, i.e. the [1024, 1024]
amplitude matrix (high 10 bits x low 10 bits) factors exactly as
    state = A @ B,   A: [1024, 4] complex,  B: [4, 1024] complex.

Host work (tiny, O(2^10 * 16)): build A and B from the 160 params.
Device work (8 cores, SPMD, no collectives):
  - materialize the two states as K=8 real matmuls producing interleaved
    (re, im) f32 pairs == complex64 memory layout, 2 MB written per core
    (cores 0-3: part 0 row-slices, cores 4-7: part 1 row-slices)
  - Pauli-Z expectation values via 4x4 Gram matrices:
        e_t = sum_ab Re( (A^T S_h conj(A))_ab * (B S_l B^H)_ab )
    computed with K=1024 PE matmuls over host-precomputed sign weights,
    then the coefficient-weighted eigenvalue (redundantly on every core).
"""

import numpy as np

import concourse.bacc as bacc
import concourse.mybir as mybir
from concourse.bass_utils import run_bass_kernel_spmd
from concourse.tile import TileContext

SUB_Q = 20
DEPTH = 2
HALF = SUB_Q // 2          # 10 qubits per A/B side
NH = 1 << HALF             # 1024
NUM_PART = 2
N_TERMS = 6
PARAMS_PER_PART = 2 * SUB_Q * DEPTH  # 80
N_CORES = 8
SLICES = 4                 # h-row slices per part (one per core)
MROWS = NH // SLICES       # 256 rows per core
F32 = mybir.dt.float32

LAST_RUN = None  # BassKernelResults of the most recent device run (for test.py)
_NC_CACHE = None


# ---------------------------------------------------------------- host math
def _part_AB(theta):
    """theta [80] float64 -> A [1024, 4], B [4, 1024] complex128 with
    state[z] = A[z >> 10] @ B[:, z & 1023]."""
    th = np.asarray(theta, np.float64).reshape(DEPTH, 2, SUB_Q)
    a0, b0 = th[0, 0] * 0.5, th[0, 1] * 0.5
    u = np.stack([np.exp(-1j * b0) * np.cos(a0),
                  np.exp(1j * b0) * np.sin(a0)], axis=1)          # [20, 2]
    a1, b1 = th[1, 0] * 0.5, th[1, 1] * 0.5
    c, s = np.cos(a1), np.sin(a1)
    g = np.empty((SUB_Q, 2, 2), np.complex128)
    g[:, 0, 0] = np.exp(-1j * b1) * c
    g[:, 0, 1] = np.exp(-1j * b1) * (-s)
    g[:, 1, 0] = np.exp(1j * b1) * s
    g[:, 1, 1] = np.exp(1j * b1) * c

    # T_t(w)[al, be] = g[t, w, be] * u[t, be ^ al]
    # S_t(z)[(al, cb), (be, cb')] = T_t(z ^ cb)[al, be] * (cb' == z)
    S = np.zeros((SUB_Q, 2, 4, 4), np.complex128)
    for t in range(SUB_Q):
        for z in range(2):
            for al in range(2):
                for cb in range(2):
                    w = z ^ cb
                    for be in range(2):
                        S[t, z, al * 2 + cb, be * 2 + z] = g[t, w, be] * u[t, be ^ al]

    cur = np.zeros((1, 4), np.complex128)
    cur[0, 0] = 1.0                                   # (al=0, carried bit 0)
    for t in range(HALF):                             # bits z_0 (MSB) .. z_9
        n = cur.shape[0]
        nxt = np.empty((2 * n, 4), np.complex128)
        nxt[0::2] = cur @ S[t, 0]
        nxt[1::2] = cur @ S[t, 1]
        cur = nxt
    A = cur                                           # [1024, 4]

    cur = np.ones((4, 1), np.complex128)
    for k in range(HALF):                             # bits z_19 .. z_10
        t = SUB_Q - 1 - k
        n = cur.shape[1]
        nxt = np.empty((4, 2 * n), np.complex128)
        nxt[:, 0:n] = S[t, 0] @ cur
        nxt[:, n:2 * n] = S[t, 1] @ cur
        cur = nxt
    B = cur                                           # [4, 1024]
    return A, B


def _parity_signs(mask):
    """(-1)^popcount(i & mask) for i in [0, 1024), as float64 [1024]."""
    idx = np.arange(NH)
    r = idx & int(mask)
    p = np.zeros_like(idx)
    for b in range(HALF):
        p ^= (r >> b) & 1
    return (1.0 - 2.0 * p).astype(np.float64)


def _chunked(x):
    """[1024, F] -> [128, 8 * F] with h = chunk * 128 + partition."""
    f = x.shape[1]
    return np.ascontiguousarray(
        x.reshape(8, 128, f).transpose(1, 0, 2).reshape(128, 8 * f)
    ).astype(np.float32)


# ---------------------------------------------------------------- device IR
def _build_nc():
    nc = bacc.Bacc("TRN2", target_bir_lowering=False, debug=False,
                   enable_asserts=False)

    d_lhsT = nc.declare_dram_parameter("lhsT", [8, MROWS], F32, isOutput=False)
    d_rhsB = nc.declare_dram_parameter("rhsB", [8, 2 * NH], F32, isOutput=False)
    d_apk = nc.declare_dram_parameter("apk", [128, 128], F32, isOutput=False)
    d_wa = nc.declare_dram_parameter("wa", [128, 1024], F32, isOutput=False)
    d_bpk = nc.declare_dram_parameter("bpk", [128, 128], F32, isOutput=False)
    d_wb = nc.declare_dram_parameter("wb", [128, 1024], F32, isOutput=False)
    d_tmat = nc.declare_dram_parameter("tmat", [64, N_TERMS], F32, isOutput=False)
    d_coefs = nc.declare_dram_parameter("coefs", [N_TERMS, 1], F32, isOutput=False)
    d_y = nc.declare_dram_parameter("y", [MROWS, 2 * NH], F32, isOutput=True)
    d_eig = nc.declare_dram_parameter("eig", [1, 1], F32, isOutput=True)

    with TileContext(nc) as tc:
        with (
            tc.tile_pool(name="cpool", bufs=1) as cpool,
            tc.tile_pool(name="ypool", bufs=2) as ypool,
            tc.tile_pool(name="ypsum", bufs=2, space="PSUM") as ypsum_pool,
            tc.tile_pool(name="spsum", bufs=1, space="PSUM") as spsum_pool,
        ):
            # ---- load inputs
            sb_lhsT = cpool.tile([8, MROWS], F32)
            nc.sync.dma_start(sb_lhsT[:], d_lhsT[:])
            sb_rhsB = cpool.tile([8, 2 * NH], F32)
            nc.sync.dma_start(sb_rhsB[:], d_rhsB[:])
            sb_apk = cpool.tile([128, 128], F32)
            nc.sync.dma_start(sb_apk[:], d_apk[:])
            sb_wa = cpool.tile([128, 1024], F32)
            nc.sync.dma_start(sb_wa[:], d_wa[:])
            sb_bpk = cpool.tile([128, 128], F32)
            nc.sync.dma_start(sb_bpk[:], d_bpk[:])
            sb_wb = cpool.tile([128, 1024], F32)
            nc.sync.dma_start(sb_wb[:], d_wb[:])
            sb_tmat = cpool.tile([64, N_TERMS], F32)
            nc.sync.dma_start(sb_tmat[:], d_tmat[:])
            sb_coefs = cpool.tile([N_TERMS, 1], F32)
            nc.sync.dma_start(sb_coefs[:], d_coefs[:])

            # ---- state materialization: y[mm, 2l + c] = (A @ B) interleaved
            for m in range(MROWS // 128):
                y_sb = ypool.tile([128, 2 * NH], F32, tag="y_sb")
                for n in range(4):
                    yp = ypsum_pool.tile([128, 512], F32, tag="yp")
                    nc.tensor.matmul(
                        yp[:],
                        sb_lhsT[:, m * 128:(m + 1) * 128],
                        sb_rhsB[:, n * 512:(n + 1) * 512],
                        start=True, stop=True,
                    )
                    dst = y_sb[:, n * 512:(n + 1) * 512]
                    if n % 2 == 0:
                        nc.vector.tensor_copy(dst, yp[:])
                    else:
                        nc.scalar.copy(dst, yp[:])
                nc.sync.dma_start(d_y[m * 128:(m + 1) * 128, :], y_sb[:])

            # ---- expectation values via Gram matrices
            # Two matmuls per side (one per plane pl of the left factor):
            # M_pl[(q, a, t) partitions, (q', pl', be) free] = sum_h wa_pl * apk
            # partition index within each = q*32 + a*6 + t (24..32 unused)
            m_sb = {}
            for (side, lhs, rhs) in (("a", sb_wa, sb_apk), ("b", sb_wb, sb_bpk)):
                for pl in range(2):
                    mp = spsum_pool.tile([64, 16], F32, tag=f"m{side}{pl}")
                    for c in range(8):
                        nc.tensor.matmul(
                            mp[:],
                            lhs[:, c * 128 + pl * 64: c * 128 + pl * 64 + 64],
                            rhs[:, c * 16:(c + 1) * 16],
                            start=(c == 0), stop=(c == 7))
                    msb = cpool.tile([64, 16], F32, tag=f"ms{side}{pl}")
                    nc.vector.tensor_copy(msb[:], mp[:])
                    m_sb[side, pl] = msb

            e_sb = []
            for q in range(NUM_PART):
                p0 = q * 32          # q block base (32-aligned)
                sl = slice(p0, p0 + 24)
                fr = q * 8           # (q, pl'=0) free block
                fi = q * 8 + 4       # (q, pl'=1) free block
                ma0, ma1 = m_sb["a", 0], m_sb["a", 1]
                mb0, mb1 = m_sb["b", 0], m_sb["b", 1]
                mare = cpool.tile([64, 4], F32, tag=f"mare{q}")
                nc.vector.tensor_add(mare[sl, :], ma0[sl, fr:fr + 4],
                                     ma1[sl, fi:fi + 4])
                maim = cpool.tile([64, 4], F32, tag=f"maim{q}")
                nc.vector.tensor_sub(maim[sl, :], ma1[sl, fr:fr + 4],
                                     ma0[sl, fi:fi + 4])
                mbre = cpool.tile([64, 4], F32, tag=f"mbre{q}")
                nc.vector.tensor_add(mbre[sl, :], mb0[sl, fr:fr + 4],
                                     mb1[sl, fi:fi + 4])
                mbim = cpool.tile([64, 4], F32, tag=f"mbim{q}")
                nc.vector.tensor_sub(mbim[sl, :], mb1[sl, fr:fr + 4],
                                     mb0[sl, fi:fi + 4])

                t1 = cpool.tile([64, 4], F32, tag=f"t1{q}")
                nc.vector.tensor_mul(t1[sl, :], mare[sl, :], mbre[sl, :])
                t2 = cpool.tile([64, 4], F32, tag=f"t2{q}")
                nc.vector.tensor_mul(t2[sl, :], maim[sl, :], mbim[sl, :])
                dd = cpool.tile([64, 4], F32, tag=f"dd{q}")
                nc.vector.tensor_sub(dd[sl, :], t1[sl, :], t2[sl, :])
                ddr = cpool.tile([64, 1], F32, tag=f"ddr{q}")
                nc.vector.reduce_sum(ddr[sl, :], dd[sl, :],
                                     axis=mybir.AxisListType.X)

                # e_q[t] = sum_a ddr[q*32 + a*6 + t]  via indicator matmul
                e_ps = spsum_pool.tile([N_TERMS, 1], F32, tag="esm")
                nc.tensor.matmul(e_ps[:], sb_tmat[sl, :], ddr[sl, :],
                                 start=True, stop=True)
                e_q = cpool.tile([N_TERMS, 1], F32, tag=f"e{q}")
                nc.vector.tensor_copy(e_q[:], e_ps[:])
                e_sb.append(e_q)

            ec = cpool.tile([N_TERMS, 1], F32)
            nc.vector.tensor_mul(ec[:], e_sb[0][:], e_sb[1][:])
            ec2 = cpool.tile([N_TERMS, 1], F32)
            nc.vector.tensor_mul(ec2[:], ec[:], sb_coefs[:])
            ones = cpool.tile([N_TERMS, 1], F32)
            nc.vector.memset(ones[:], 1.0)
            eig_ps = spsum_pool.tile([1, 1], F32, tag="esm")
            nc.tensor.matmul(eig_ps[:], ones[:], ec2[:], start=True, stop=True)
            eig_sb = cpool.tile([1, 1], F32)
            nc.vector.tensor_copy(eig_sb[:], eig_ps[:])
            nc.sync.dma_start(d_eig[:], eig_sb[:])

    nc.finalize()
    return nc


# ---------------------------------------------------------------- entry
def kernel(params, coefs, z_masks):
    global LAST_RUN, _NC_CACHE
    params = np.asarray(params, np.float64)
    coefs = np.asarray(coefs, np.float32)
    z_masks = np.asarray(z_masks)

    AB = [_part_AB(params[p * PARAMS_PER_PART:(p + 1) * PARAMS_PER_PART])
          for p in range(NUM_PART)]

    # shared (all-core) expectation inputs
    # apk free layout: a' = q*8 + pl'*4 + be
    # wa partition layout: m = pl*64 + q*32 + a*6 + t  (zeros at 24..32)
    apk = np.empty((NH, 16))
    wa = np.zeros((NH, 2, 2, 32))
    bpk = np.empty((NH, 16))
    wb = np.zeros((NH, 2, 2, 32))
    for q in range(NUM_PART):
        A, B = AB[q]
        Bt = B.T
        for pl, plane in enumerate((A.real, A.imag)):
            apk[:, q * 8 + pl * 4: q * 8 + pl * 4 + 4] = plane
        for pl, plane in enumerate((Bt.real, Bt.imag)):
            bpk[:, q * 8 + pl * 4: q * 8 + pl * 4 + 4] = plane
        sh = np.stack([_parity_signs(int(z_masks[q, t]) >> HALF)
                       for t in range(N_TERMS)])          # [6, 1024]
        sl = np.stack([_parity_signs(int(z_masks[q, t]) & (NH - 1))
                       for t in range(N_TERMS)])
        # [h, a, t] -> flattened a*6 + t within the 32-wide (pl, q) block
        wa[:, 0, q, :24] = (A.real[:, :, None] * sh.T[:, None, :]).reshape(NH, 24)
        wa[:, 1, q, :24] = (A.imag[:, :, None] * sh.T[:, None, :]).reshape(NH, 24)
        wb[:, 0, q, :24] = (Bt.real[:, :, None] * sl.T[:, None, :]).reshape(NH, 24)
        wb[:, 1, q, :24] = (Bt.imag[:, :, None] * sl.T[:, None, :]).reshape(NH, 24)

    apk_c = _chunked(apk)
    wa_c = _chunked(wa.reshape(NH, 128))
    bpk_c = _chunked(bpk)
    wb_c = _chunked(wb.reshape(NH, 128))
    coefs_in = coefs.reshape(N_TERMS, 1).astype(np.float32)
    tmat = np.zeros((64, N_TERMS), np.float32)
    for q in range(2):
        for a in range(4):
            for t in range(N_TERMS):
                tmat[q * 32 + a * N_TERMS + t, t] = 1.0

    in_maps = []
    for core in range(N_CORES):
        p, j = divmod(core, SLICES)
        A, B = AB[p]
        h0 = j * MROWS
        lhsT = np.empty((8, MROWS), np.float32)
        lhsT[0:4] = A.real[h0:h0 + MROWS].T
        lhsT[4:8] = A.imag[h0:h0 + MROWS].T
        rhsB = np.empty((8, 2 * NH), np.float32)
        rhsB[0:4, 0::2] = B.real
        rhsB[4:8, 0::2] = -B.imag
        rhsB[0:4, 1::2] = B.imag
        rhsB[4:8, 1::2] = B.real
        in_maps.append({
            "lhsT": lhsT, "rhsB": rhsB, "apk": apk_c, "wa": wa_c,
            "bpk": bpk_c, "wb": wb_c, "tmat": tmat, "coefs": coefs_in,
        })

    if _NC_CACHE is None:
        _NC_CACHE = _build_nc()
    res = run_bass_kernel_spmd(_NC_CACHE, in_maps, list(range(N_CORES)))
    LAST_RUN = res

    states = []
    for p in range(NUM_PART):
        rows = np.concatenate([res.results[p * SLICES + j]["y"]
                               for j in range(SLICES)], axis=0)  # [1024, 2048]
        states.append(np.ascontiguousarray(rows).view(np.complex64).reshape(NH * NH))
    eig = np.float32(res.results[0]["eig"][0, 0])
    return (np.asarray(eig, np.float32), states[0], states[1])


# revision 18
# speedup vs baseline: 1.2948x; 1.2948x over previous
"""Distributed VQE kernel for 8 TRN2 NeuronCores.

Math: each part's circuit is
    |psi> = P . U2 . P . U1 |0>
where U1/U2 are tensor products of single-qubit RZ*RY gates and P is the
CNOT-ladder basis permutation |x> -> |prefix_xor(x)>.  U1|0> is a product
state; relabeling through P keeps a Markov (bond-2) structure, applying U2
keeps bond 2, and the final P relabeling lifts it to bond 4.  So the full
2^20-amplitude state is an MPS of bond dimension 4, i.e. the [1024, 1024]
amplitude matrix (high 10 bits x low 10 bits) factors exactly as
    state = A @ B,   A: [1024, 4] complex,  B: [4, 1024] complex.

Host work (tiny, O(2^10 * 16)): build A and B from the 160 params.
Device work (8 cores, SPMD, no collectives):
  - materialize the two states as K=8 real matmuls producing interleaved
    (re, im) f32 pairs == complex64 memory layout, 2 MB written per core
    (cores 0-3: part 0 row-slices, cores 4-7: part 1 row-slices)
  - Pauli-Z expectation values via 4x4 Gram matrices:
        e_t = sum_ab Re( (A^T S_h conj(A))_ab * (B S_l B^H)_ab )
    computed with K=1024 PE matmuls over host-precomputed sign weights,
    then the coefficient-weighted eigenvalue (redundantly on every core).
"""

import numpy as np

import concourse.bacc as bacc
import concourse.mybir as mybir
from concourse.bass_utils import run_bass_kernel_spmd
from concourse.tile import TileContext

SUB_Q = 20
DEPTH = 2
HALF = SUB_Q // 2          # 10 qubits per A/B side
NH = 1 << HALF             # 1024
NUM_PART = 2
N_TERMS = 6
PARAMS_PER_PART = 2 * SUB_Q * DEPTH  # 80
N_CORES = 8
SLICES = 4                 # h-row slices per part (one per core)
MROWS = NH // SLICES       # 256 rows per core
F32 = mybir.dt.float32
F32R = mybir.dt.float32r   # single-pass PE fp32 (1 cyc/row at N>=256)

LAST_RUN = None  # BassKernelResults of the most recent device run (for test.py)
_NC_CACHE = None


# ---------------------------------------------------------------- host math
def _part_AB(theta):
    """theta [80] float64 -> A [1024, 4], B [4, 1024] complex128 with
    state[z] = A[z >> 10] @ B[:, z & 1023]."""
    th = np.asarray(theta, np.float64).reshape(DEPTH, 2, SUB_Q)
    a0, b0 = th[0, 0] * 0.5, th[0, 1] * 0.5
    u = np.stack([np.exp(-1j * b0) * np.cos(a0),
                  np.exp(1j * b0) * np.sin(a0)], axis=1)          # [20, 2]
    a1, b1 = th[1, 0] * 0.5, th[1, 1] * 0.5
    c, s = np.cos(a1), np.sin(a1)
    g = np.empty((SUB_Q, 2, 2), np.complex128)
    g[:, 0, 0] = np.exp(-1j * b1) * c
    g[:, 0, 1] = np.exp(-1j * b1) * (-s)
    g[:, 1, 0] = np.exp(1j * b1) * s
    g[:, 1, 1] = np.exp(1j * b1) * c

    # T_t(w)[al, be] = g[t, w, be] * u[t, be ^ al]
    # S_t(z)[(al, cb), (be, cb')] = T_t(z ^ cb)[al, be] * (cb' == z)
    S = np.zeros((SUB_Q, 2, 4, 4), np.complex128)
    for t in range(SUB_Q):
        for z in range(2):
            for al in range(2):
                for cb in range(2):
                    w = z ^ cb
                    for be in range(2):
                        S[t, z, al * 2 + cb, be * 2 + z] = g[t, w, be] * u[t, be ^ al]

    cur = np.zeros((1, 4), np.complex128)
    cur[0, 0] = 1.0                                   # (al=0, carried bit 0)
    for t in range(HALF):                             # bits z_0 (MSB) .. z_9
        n = cur.shape[0]
        nxt = np.empty((2 * n, 4), np.complex128)
        nxt[0::2] = cur @ S[t, 0]
        nxt[1::2] = cur @ S[t, 1]
        cur = nxt
    A = cur                                           # [1024, 4]

    cur = np.ones((4, 1), np.complex128)
    for k in range(HALF):                             # bits z_19 .. z_10
        t = SUB_Q - 1 - k
        n = cur.shape[1]
        nxt = np.empty((4, 2 * n), np.complex128)
        nxt[:, 0:n] = S[t, 0] @ cur
        nxt[:, n:2 * n] = S[t, 1] @ cur
        cur = nxt
    B = cur                                           # [4, 1024]
    return A, B


def _parity_signs(mask):
    """(-1)^popcount(i & mask) for i in [0, 1024), as float64 [1024]."""
    idx = np.arange(NH)
    r = idx & int(mask)
    p = np.zeros_like(idx)
    for b in range(HALF):
        p ^= (r >> b) & 1
    return (1.0 - 2.0 * p).astype(np.float64)


def _chunked(x):
    """[1024, F] -> [128, 8 * F] with h = chunk * 128 + partition."""
    f = x.shape[1]
    return np.ascontiguousarray(
        x.reshape(8, 128, f).transpose(1, 0, 2).reshape(128, 8 * f)
    ).astype(np.float32)


# ---------------------------------------------------------------- device IR
def _build_nc():
    nc = bacc.Bacc("TRN2", target_bir_lowering=False, debug=False,
                   enable_asserts=False)

    d_lhsT = nc.declare_dram_parameter("lhsT", [8, MROWS], F32R, isOutput=False)
    d_rhsB = nc.declare_dram_parameter("rhsB", [8, 2 * NH], F32R, isOutput=False)
    d_apk = nc.declare_dram_parameter("apk", [128, 128], F32, isOutput=False)
    d_wa = nc.declare_dram_parameter("wa", [128, 1024], F32, isOutput=False)
    d_bpk = nc.declare_dram_parameter("bpk", [128, 128], F32, isOutput=False)
    d_wb = nc.declare_dram_parameter("wb", [128, 1024], F32, isOutput=False)
    d_tmat = nc.declare_dram_parameter("tmat", [64, N_TERMS], F32, isOutput=False)
    d_coefs = nc.declare_dram_parameter("coefs", [N_TERMS, 1], F32, isOutput=False)
    d_y = nc.declare_dram_parameter("y", [MROWS, 2 * NH], F32, isOutput=True)
    d_eig = nc.declare_dram_parameter("eig", [1, 1], F32, isOutput=True)

    with TileContext(nc) as tc:
        with (
            tc.tile_pool(name="cpool", bufs=1) as cpool,
            tc.tile_pool(name="ypool", bufs=2) as ypool,
            tc.tile_pool(name="ypsum", bufs=3, space="PSUM") as ypsum_pool,
            tc.tile_pool(name="spsum", bufs=1, space="PSUM") as spsum_pool,
        ):
            # ---- load inputs
            sb_lhsT = cpool.tile([8, MROWS], F32R)
            nc.sync.dma_start(sb_lhsT[:], d_lhsT[:])
            sb_rhsB = cpool.tile([8, 2 * NH], F32R)
            nc.sync.dma_start(sb_rhsB[:], d_rhsB[:])
            sb_apk = cpool.tile([128, 128], F32)
            nc.sync.dma_start(sb_apk[:], d_apk[:])
            sb_wa = cpool.tile([128, 1024], F32)
            nc.sync.dma_start(sb_wa[:], d_wa[:])
            sb_bpk = cpool.tile([128, 128], F32)
            nc.sync.dma_start(sb_bpk[:], d_bpk[:])
            sb_wb = cpool.tile([128, 1024], F32)
            nc.sync.dma_start(sb_wb[:], d_wb[:])
            sb_tmat = cpool.tile([64, N_TERMS], F32)
            nc.sync.dma_start(sb_tmat[:], d_tmat[:])
            sb_coefs = cpool.tile([N_TERMS, 1], F32)
            nc.sync.dma_start(sb_coefs[:], d_coefs[:])

            # ---- state materialization: y[mm, 2l + c] = (A @ B) interleaved
            for m in range(MROWS // 128):
                y_sb = ypool.tile([128, 2 * NH], F32, tag="y_sb")
                for n in range(4):
                    yp = ypsum_pool.tile([128, 512], F32, tag="yp")
                    nc.tensor.matmul(
                        yp[:],
                        sb_lhsT[:, m * 128:(m + 1) * 128],
                        sb_rhsB[:, n * 512:(n + 1) * 512],
                        start=True, stop=True,
                    )
                    dst = y_sb[:, n * 512:(n + 1) * 512]
                    nc.vector.tensor_copy(dst, yp[:])
                    nc.sync.dma_start(
                        d_y[m * 128:(m + 1) * 128, n * 512:(n + 1) * 512], dst)

            # ---- expectation values via Gram matrices
            # Two matmuls per side (one per plane pl of the left factor):
            # M_pl[(q, a, t) partitions, (q', pl', be) free] = sum_h wa_pl * apk
            # partition index within each = q*32 + a*6 + t (24..32 unused)
            m_sb = {}
            for (side, lhs, rhs) in (("a", sb_wa, sb_apk), ("b", sb_wb, sb_bpk)):
                for pl in range(2):
                    mp = spsum_pool.tile([64, 16], F32, tag=f"m{side}{pl}")
                    for c in range(8):
                        nc.tensor.matmul(
                            mp[:],
                            lhs[:, c * 128 + pl * 64: c * 128 + pl * 64 + 64],
                            rhs[:, c * 16:(c + 1) * 16],
                            start=(c == 0), stop=(c == 7))
                    msb = cpool.tile([64, 16], F32, tag=f"ms{side}{pl}")
                    nc.vector.tensor_copy(msb[:], mp[:])
                    m_sb[side, pl] = msb

            e_sb = []
            for q in range(NUM_PART):
                p0 = q * 32          # q block base (32-aligned)
                sl = slice(p0, p0 + 24)
                fr = q * 8           # (q, pl'=0) free block
                fi = q * 8 + 4       # (q, pl'=1) free block
                ma0, ma1 = m_sb["a", 0], m_sb["a", 1]
                mb0, mb1 = m_sb["b", 0], m_sb["b", 1]
                mare = cpool.tile([64, 4], F32, tag=f"mare{q}")
                nc.vector.tensor_add(mare[sl, :], ma0[sl, fr:fr + 4],
                                     ma1[sl, fi:fi + 4])
                maim = cpool.tile([64, 4], F32, tag=f"maim{q}")
                nc.vector.tensor_sub(maim[sl, :], ma1[sl, fr:fr + 4],
                                     ma0[sl, fi:fi + 4])
                mbre = cpool.tile([64, 4], F32, tag=f"mbre{q}")
                nc.vector.tensor_add(mbre[sl, :], mb0[sl, fr:fr + 4],
                                     mb1[sl, fi:fi + 4])
                mbim = cpool.tile([64, 4], F32, tag=f"mbim{q}")
                nc.vector.tensor_sub(mbim[sl, :], mb1[sl, fr:fr + 4],
                                     mb0[sl, fi:fi + 4])

                t1 = cpool.tile([64, 4], F32, tag=f"t1{q}")
                nc.vector.tensor_mul(t1[sl, :], mare[sl, :], mbre[sl, :])
                t2 = cpool.tile([64, 4], F32, tag=f"t2{q}")
                nc.vector.tensor_mul(t2[sl, :], maim[sl, :], mbim[sl, :])
                dd = cpool.tile([64, 4], F32, tag=f"dd{q}")
                nc.vector.tensor_sub(dd[sl, :], t1[sl, :], t2[sl, :])
                ddr = cpool.tile([64, 1], F32, tag=f"ddr{q}")
                nc.vector.reduce_sum(ddr[sl, :], dd[sl, :],
                                     axis=mybir.AxisListType.X)

                # e_q[t] = sum_a ddr[q*32 + a*6 + t]  via indicator matmul
                e_ps = spsum_pool.tile([N_TERMS, 1], F32, tag="esm")
                nc.tensor.matmul(e_ps[:], sb_tmat[sl, :], ddr[sl, :],
                                 start=True, stop=True)
                e_q = cpool.tile([N_TERMS, 1], F32, tag=f"e{q}")
                nc.vector.tensor_copy(e_q[:], e_ps[:])
                e_sb.append(e_q)

            ec = cpool.tile([N_TERMS, 1], F32)
            nc.vector.tensor_mul(ec[:], e_sb[0][:], e_sb[1][:])
            ec2 = cpool.tile([N_TERMS, 1], F32)
            nc.vector.tensor_mul(ec2[:], ec[:], sb_coefs[:])
            ones = cpool.tile([N_TERMS, 1], F32)
            nc.vector.memset(ones[:], 1.0)
            eig_ps = spsum_pool.tile([1, 1], F32, tag="esm")
            nc.tensor.matmul(eig_ps[:], ones[:], ec2[:], start=True, stop=True)
            eig_sb = cpool.tile([1, 1], F32)
            nc.vector.tensor_copy(eig_sb[:], eig_ps[:])
            nc.sync.dma_start(d_eig[:], eig_sb[:])

    nc.finalize()
    return nc


# ---------------------------------------------------------------- entry
def kernel(params, coefs, z_masks):
    global LAST_RUN, _NC_CACHE
    params = np.asarray(params, np.float64)
    coefs = np.asarray(coefs, np.float32)
    z_masks = np.asarray(z_masks)

    AB = [_part_AB(params[p * PARAMS_PER_PART:(p + 1) * PARAMS_PER_PART])
          for p in range(NUM_PART)]

    # shared (all-core) expectation inputs
    # apk free layout: a' = q*8 + pl'*4 + be
    # wa partition layout: m = pl*64 + q*32 + a*6 + t  (zeros at 24..32)
    apk = np.empty((NH, 16))
    wa = np.zeros((NH, 2, 2, 32))
    bpk = np.empty((NH, 16))
    wb = np.zeros((NH, 2, 2, 32))
    for q in range(NUM_PART):
        A, B = AB[q]
        Bt = B.T
        for pl, plane in enumerate((A.real, A.imag)):
            apk[:, q * 8 + pl * 4: q * 8 + pl * 4 + 4] = plane
        for pl, plane in enumerate((Bt.real, Bt.imag)):
            bpk[:, q * 8 + pl * 4: q * 8 + pl * 4 + 4] = plane
        sh = np.stack([_parity_signs(int(z_masks[q, t]) >> HALF)
                       for t in range(N_TERMS)])          # [6, 1024]
        sl = np.stack([_parity_signs(int(z_masks[q, t]) & (NH - 1))
                       for t in range(N_TERMS)])
        # [h, a, t] -> flattened a*6 + t within the 32-wide (pl, q) block
        wa[:, 0, q, :24] = (A.real[:, :, None] * sh.T[:, None, :]).reshape(NH, 24)
        wa[:, 1, q, :24] = (A.imag[:, :, None] * sh.T[:, None, :]).reshape(NH, 24)
        wb[:, 0, q, :24] = (Bt.real[:, :, None] * sl.T[:, None, :]).reshape(NH, 24)
        wb[:, 1, q, :24] = (Bt.imag[:, :, None] * sl.T[:, None, :]).reshape(NH, 24)

    apk_c = _chunked(apk)
    wa_c = _chunked(wa.reshape(NH, 128))
    bpk_c = _chunked(bpk)
    wb_c = _chunked(wb.reshape(NH, 128))
    coefs_in = coefs.reshape(N_TERMS, 1).astype(np.float32)
    tmat = np.zeros((64, N_TERMS), np.float32)
    for q in range(2):
        for a in range(4):
            for t in range(N_TERMS):
                tmat[q * 32 + a * N_TERMS + t, t] = 1.0

    in_maps = []
    for core in range(N_CORES):
        p, j = divmod(core, SLICES)
        A, B = AB[p]
        h0 = j * MROWS
        lhsT = np.empty((8, MROWS), np.float32)
        lhsT[0:4] = A.real[h0:h0 + MROWS].T
        lhsT[4:8] = A.imag[h0:h0 + MROWS].T
        rhsB = np.empty((8, 2 * NH), np.float32)
        rhsB[0:4, 0::2] = B.real
        rhsB[4:8, 0::2] = -B.imag
        rhsB[0:4, 1::2] = B.imag
        rhsB[4:8, 1::2] = B.real
        in_maps.append({
            "lhsT": lhsT, "rhsB": rhsB, "apk": apk_c, "wa": wa_c,
            "bpk": bpk_c, "wb": wb_c, "tmat": tmat, "coefs": coefs_in,
        })

    if _NC_CACHE is None:
        _NC_CACHE = _build_nc()
    res = run_bass_kernel_spmd(_NC_CACHE, in_maps, list(range(N_CORES)))
    LAST_RUN = res

    states = []
    for p in range(NUM_PART):
        rows = np.concatenate([res.results[p * SLICES + j]["y"]
                               for j in range(SLICES)], axis=0)  # [1024, 2048]
        states.append(np.ascontiguousarray(rows).view(np.complex64).reshape(NH * NH))
    eig = np.float32(res.results[0]["eig"][0, 0])
    return (np.asarray(eig, np.float32), states[0], states[1])


# revision 22
# speedup vs baseline: 1.4942x; 1.1540x over previous
"""Distributed VQE kernel for 8 TRN2 NeuronCores.

Math: each part's circuit is
    |psi> = P . U2 . P . U1 |0>
where U1/U2 are tensor products of single-qubit RZ*RY gates and P is the
CNOT-ladder basis permutation |x> -> |prefix_xor(x)>.  U1|0> is a product
state; relabeling through P keeps a Markov (bond-2) structure, applying U2
keeps bond 2, and the final P relabeling lifts it to bond 4.  So the full
2^20-amplitude state is an MPS of bond dimension 4, i.e. the [1024, 1024]
amplitude matrix (high 10 bits x low 10 bits) factors exactly as
    state = A @ B,   A: [1024, 4] complex,  B: [4, 1024] complex.

Host work (tiny, O(2^10 * 16)): build A and B from the 160 params.
Device work (8 cores, SPMD, no collectives):
  - materialize the two states as K=8 real matmuls producing interleaved
    (re, im) f32 pairs == complex64 memory layout, 2 MB written per core
    (cores 0-3: part 0 row-slices, cores 4-7: part 1 row-slices)
  - Pauli-Z expectation values via 4x4 Gram matrices:
        e_t = sum_ab Re( (A^T S_h conj(A))_ab * (B S_l B^H)_ab )
    computed with K=1024 PE matmuls over host-precomputed sign weights,
    then the coefficient-weighted eigenvalue (redundantly on every core).
"""

import numpy as np

import concourse.bacc as bacc
import concourse.mybir as mybir
from concourse.bass_utils import run_bass_kernel_spmd
from concourse.tile import TileContext

SUB_Q = 20
DEPTH = 2
HALF = SUB_Q // 2          # 10 qubits per A/B side
NH = 1 << HALF             # 1024
NUM_PART = 2
N_TERMS = 6
PARAMS_PER_PART = 2 * SUB_Q * DEPTH  # 80
N_CORES = 8
SLICES = 4                 # h-row slices per part (one per core)
MROWS = NH // SLICES       # 256 rows per core
F32 = mybir.dt.float32
F32R = mybir.dt.float32r   # single-pass PE fp32 (1 cyc/row at N>=256)

LAST_RUN = None  # BassKernelResults of the most recent device run (for test.py)
_NC_CACHE = None


# ---------------------------------------------------------------- host math
def _part_AB(theta):
    """theta [80] float64 -> A [1024, 4], B [4, 1024] complex128 with
    state[z] = A[z >> 10] @ B[:, z & 1023]."""
    th = np.asarray(theta, np.float64).reshape(DEPTH, 2, SUB_Q)
    a0, b0 = th[0, 0] * 0.5, th[0, 1] * 0.5
    u = np.stack([np.exp(-1j * b0) * np.cos(a0),
                  np.exp(1j * b0) * np.sin(a0)], axis=1)          # [20, 2]
    a1, b1 = th[1, 0] * 0.5, th[1, 1] * 0.5
    c, s = np.cos(a1), np.sin(a1)
    g = np.empty((SUB_Q, 2, 2), np.complex128)
    g[:, 0, 0] = np.exp(-1j * b1) * c
    g[:, 0, 1] = np.exp(-1j * b1) * (-s)
    g[:, 1, 0] = np.exp(1j * b1) * s
    g[:, 1, 1] = np.exp(1j * b1) * c

    # T_t(w)[al, be] = g[t, w, be] * u[t, be ^ al]
    # S_t(z)[(al, cb), (be, cb')] = T_t(z ^ cb)[al, be] * (cb' == z)
    S = np.zeros((SUB_Q, 2, 4, 4), np.complex128)
    for t in range(SUB_Q):
        for z in range(2):
            for al in range(2):
                for cb in range(2):
                    w = z ^ cb
                    for be in range(2):
                        S[t, z, al * 2 + cb, be * 2 + z] = g[t, w, be] * u[t, be ^ al]

    cur = np.zeros((1, 4), np.complex128)
    cur[0, 0] = 1.0                                   # (al=0, carried bit 0)
    for t in range(HALF):                             # bits z_0 (MSB) .. z_9
        n = cur.shape[0]
        nxt = np.empty((2 * n, 4), np.complex128)
        nxt[0::2] = cur @ S[t, 0]
        nxt[1::2] = cur @ S[t, 1]
        cur = nxt
    A = cur                                           # [1024, 4]

    cur = np.ones((4, 1), np.complex128)
    for k in range(HALF):                             # bits z_19 .. z_10
        t = SUB_Q - 1 - k
        n = cur.shape[1]
        nxt = np.empty((4, 2 * n), np.complex128)
        nxt[:, 0:n] = S[t, 0] @ cur
        nxt[:, n:2 * n] = S[t, 1] @ cur
        cur = nxt
    B = cur                                           # [4, 1024]
    return A, B


def _parity_signs(mask):
    """(-1)^popcount(i & mask) for i in [0, 1024), as float64 [1024]."""
    idx = np.arange(NH)
    r = idx & int(mask)
    p = np.zeros_like(idx)
    for b in range(HALF):
        p ^= (r >> b) & 1
    return (1.0 - 2.0 * p).astype(np.float64)


def _chunked(x):
    """[1024, F] -> [128, 8 * F] with h = chunk * 128 + partition."""
    f = x.shape[1]
    return np.ascontiguousarray(
        x.reshape(8, 128, f).transpose(1, 0, 2).reshape(128, 8 * f)
    ).astype(np.float32)


# ---------------------------------------------------------------- device IR
def _build_nc():
    nc = bacc.Bacc("TRN2", target_bir_lowering=False, debug=False,
                   enable_asserts=False)

    d_lhsT = nc.declare_dram_parameter("lhsT", [8, MROWS], F32R, isOutput=False)
    d_rhsB = nc.declare_dram_parameter("rhsB", [8, 2 * NH], F32R, isOutput=False)
    d_apk = nc.declare_dram_parameter("apk", [128, 128], F32R, isOutput=False)
    d_wa = nc.declare_dram_parameter("wa", [128, 1024], F32R, isOutput=False)
    d_bpk = nc.declare_dram_parameter("bpk", [128, 128], F32R, isOutput=False)
    d_wb = nc.declare_dram_parameter("wb", [128, 1024], F32R, isOutput=False)
    d_tmat = nc.declare_dram_parameter("tmat", [64, N_TERMS], F32, isOutput=False)
    d_coefs = nc.declare_dram_parameter("coefs", [N_TERMS, 1], F32, isOutput=False)
    # output state slice, stored chunk-contiguous: [m*4+n, 128, 512]
    d_y = nc.declare_dram_parameter("y", [8, 128, 512], F32, isOutput=True)
    d_eig = nc.declare_dram_parameter("eig", [1, 1], F32, isOutput=True)

    with TileContext(nc) as tc:
        with (
            tc.tile_pool(name="cpool", bufs=1) as cpool,
            tc.tile_pool(name="ypool", bufs=2) as ypool,
            tc.tile_pool(name="ypsum", bufs=3, space="PSUM") as ypsum_pool,
            tc.tile_pool(name="spsum", bufs=1, space="PSUM") as spsum_pool,
        ):
            # ---- load inputs
            sb_lhsT = cpool.tile([8, MROWS], F32R)
            nc.sync.dma_start(sb_lhsT[:], d_lhsT[:])
            sb_rhsB = cpool.tile([8, 2 * NH], F32R)
            nc.sync.dma_start(sb_rhsB[:], d_rhsB[:])
            sb_apk = cpool.tile([128, 128], F32R)
            nc.sync.dma_start(sb_apk[:], d_apk[:])
            sb_wa = cpool.tile([128, 1024], F32R)
            nc.sync.dma_start(sb_wa[:], d_wa[:])
            sb_bpk = cpool.tile([128, 128], F32R)
            nc.sync.dma_start(sb_bpk[:], d_bpk[:])
            sb_wb = cpool.tile([128, 1024], F32R)
            nc.sync.dma_start(sb_wb[:], d_wb[:])
            sb_tmat = cpool.tile([64, N_TERMS], F32)
            nc.sync.dma_start(sb_tmat[:], d_tmat[:])
            sb_coefs = cpool.tile([N_TERMS, 1], F32)
            nc.sync.dma_start(sb_coefs[:], d_coefs[:])

            # ---- state materialization: y[mm, 2l + c] = (A @ B) interleaved
            for m in range(MROWS // 128):
                y_sb = ypool.tile([128, 2 * NH], F32, tag="y_sb")
                for n in range(4):
                    yp = ypsum_pool.tile([128, 512], F32, tag="yp")
                    nc.tensor.matmul(
                        yp[:],
                        sb_lhsT[:, m * 128:(m + 1) * 128],
                        sb_rhsB[:, n * 512:(n + 1) * 512],
                        start=True, stop=True,
                    )
                    dst = y_sb[:, n * 512:(n + 1) * 512]
                    nc.vector.tensor_copy(dst, yp[:])
                    nc.sync.dma_start(d_y[m * 4 + n, :, :], dst)

            # ---- expectation values via Gram matrices
            # Two matmuls per side (one per plane pl of the left factor):
            # M_pl[(q, a, t) partitions, (q', pl', be) free] = sum_h wa_pl * apk
            # partition index within each = q*32 + a*6 + t (24..32 unused)
            m_sb = {}
            for (side, lhs, rhs) in (("a", sb_wa, sb_apk), ("b", sb_wb, sb_bpk)):
                for pl in range(2):
                    mp = spsum_pool.tile([64, 16], F32, tag=f"m{side}{pl}")
                    for c in range(8):
                        nc.tensor.matmul(
                            mp[:],
                            lhs[:, c * 128 + pl * 64: c * 128 + pl * 64 + 64],
                            rhs[:, c * 16:(c + 1) * 16],
                            start=(c == 0), stop=(c == 7))
                    msb = cpool.tile([64, 16], F32, tag=f"ms{side}{pl}")
                    nc.vector.tensor_copy(msb[:], mp[:])
                    m_sb[side, pl] = msb

            e_sb = []
            for q in range(NUM_PART):
                p0 = q * 32          # q block base (32-aligned)
                sl = slice(p0, p0 + 24)
                fr = q * 8           # (q, pl'=0) free block
                fi = q * 8 + 4       # (q, pl'=1) free block
                ma0, ma1 = m_sb["a", 0], m_sb["a", 1]
                mb0, mb1 = m_sb["b", 0], m_sb["b", 1]
                mare = cpool.tile([64, 4], F32, tag=f"mare{q}")
                nc.vector.tensor_add(mare[sl, :], ma0[sl, fr:fr + 4],
                                     ma1[sl, fi:fi + 4])
                maim = cpool.tile([64, 4], F32, tag=f"maim{q}")
                nc.vector.tensor_sub(maim[sl, :], ma1[sl, fr:fr + 4],
                                     ma0[sl, fi:fi + 4])
                mbre = cpool.tile([64, 4], F32, tag=f"mbre{q}")
                nc.vector.tensor_add(mbre[sl, :], mb0[sl, fr:fr + 4],
                                     mb1[sl, fi:fi + 4])
                mbim = cpool.tile([64, 4], F32, tag=f"mbim{q}")
                nc.vector.tensor_sub(mbim[sl, :], mb1[sl, fr:fr + 4],
                                     mb0[sl, fi:fi + 4])

                t1 = cpool.tile([64, 4], F32, tag=f"t1{q}")
                nc.vector.tensor_mul(t1[sl, :], mare[sl, :], mbre[sl, :])
                t2 = cpool.tile([64, 4], F32, tag=f"t2{q}")
                nc.vector.tensor_mul(t2[sl, :], maim[sl, :], mbim[sl, :])
                dd = cpool.tile([64, 4], F32, tag=f"dd{q}")
                nc.vector.tensor_sub(dd[sl, :], t1[sl, :], t2[sl, :])
                ddr = cpool.tile([64, 1], F32, tag=f"ddr{q}")
                nc.vector.reduce_sum(ddr[sl, :], dd[sl, :],
                                     axis=mybir.AxisListType.X)

                # e_q[t] = sum_a ddr[q*32 + a*6 + t]  via indicator matmul
                e_ps = spsum_pool.tile([N_TERMS, 1], F32, tag="esm")
                nc.tensor.matmul(e_ps[:], sb_tmat[sl, :], ddr[sl, :],
                                 start=True, stop=True)
                e_q = cpool.tile([N_TERMS, 1], F32, tag=f"e{q}")
                nc.vector.tensor_copy(e_q[:], e_ps[:])
                e_sb.append(e_q)

            ec = cpool.tile([N_TERMS, 1], F32)
            nc.vector.tensor_mul(ec[:], e_sb[0][:], e_sb[1][:])
            ec2 = cpool.tile([N_TERMS, 1], F32)
            nc.vector.tensor_mul(ec2[:], ec[:], sb_coefs[:])
            ones = cpool.tile([N_TERMS, 1], F32)
            nc.vector.memset(ones[:], 1.0)
            eig_ps = spsum_pool.tile([1, 1], F32, tag="esm")
            nc.tensor.matmul(eig_ps[:], ones[:], ec2[:], start=True, stop=True)
            eig_sb = cpool.tile([1, 1], F32)
            nc.vector.tensor_copy(eig_sb[:], eig_ps[:])
            nc.sync.dma_start(d_eig[:], eig_sb[:])

    nc.finalize()
    return nc


# ---------------------------------------------------------------- entry
def kernel(params, coefs, z_masks):
    global LAST_RUN, _NC_CACHE
    params = np.asarray(params, np.float64)
    coefs = np.asarray(coefs, np.float32)
    z_masks = np.asarray(z_masks)

    AB = [_part_AB(params[p * PARAMS_PER_PART:(p + 1) * PARAMS_PER_PART])
          for p in range(NUM_PART)]

    # shared (all-core) expectation inputs
    # apk free layout: a' = q*8 + pl'*4 + be
    # wa partition layout: m = pl*64 + q*32 + a*6 + t  (zeros at 24..32)
    apk = np.empty((NH, 16))
    wa = np.zeros((NH, 2, 2, 32))
    bpk = np.empty((NH, 16))
    wb = np.zeros((NH, 2, 2, 32))
    for q in range(NUM_PART):
        A, B = AB[q]
        Bt = B.T
        for pl, plane in enumerate((A.real, A.imag)):
            apk[:, q * 8 + pl * 4: q * 8 + pl * 4 + 4] = plane
        for pl, plane in enumerate((Bt.real, Bt.imag)):
            bpk[:, q * 8 + pl * 4: q * 8 + pl * 4 + 4] = plane
        sh = np.stack([_parity_signs(int(z_masks[q, t]) >> HALF)
                       for t in range(N_TERMS)])          # [6, 1024]
        sl = np.stack([_parity_signs(int(z_masks[q, t]) & (NH - 1))
                       for t in range(N_TERMS)])
        # [h, a, t] -> flattened a*6 + t within the 32-wide (pl, q) block
        wa[:, 0, q, :24] = (A.real[:, :, None] * sh.T[:, None, :]).reshape(NH, 24)
        wa[:, 1, q, :24] = (A.imag[:, :, None] * sh.T[:, None, :]).reshape(NH, 24)
        wb[:, 0, q, :24] = (Bt.real[:, :, None] * sl.T[:, None, :]).reshape(NH, 24)
        wb[:, 1, q, :24] = (Bt.imag[:, :, None] * sl.T[:, None, :]).reshape(NH, 24)

    apk_c = _chunked(apk)
    wa_c = _chunked(wa.reshape(NH, 128))
    bpk_c = _chunked(bpk)
    wb_c = _chunked(wb.reshape(NH, 128))
    coefs_in = coefs.reshape(N_TERMS, 1).astype(np.float32)
    tmat = np.zeros((64, N_TERMS), np.float32)
    for q in range(2):
        for a in range(4):
            for t in range(N_TERMS):
                tmat[q * 32 + a * N_TERMS + t, t] = 1.0

    in_maps = []
    for core in range(N_CORES):
        p, j = divmod(core, SLICES)
        A, B = AB[p]
        h0 = j * MROWS
        lhsT = np.empty((8, MROWS), np.float32)
        lhsT[0:4] = A.real[h0:h0 + MROWS].T
        lhsT[4:8] = A.imag[h0:h0 + MROWS].T
        rhsB = np.empty((8, 2 * NH), np.float32)
        rhsB[0:4, 0::2] = B.real
        rhsB[4:8, 0::2] = -B.imag
        rhsB[0:4, 1::2] = B.imag
        rhsB[4:8, 1::2] = B.real
        in_maps.append({
            "lhsT": lhsT, "rhsB": rhsB, "apk": apk_c, "wa": wa_c,
            "bpk": bpk_c, "wb": wb_c, "tmat": tmat, "coefs": coefs_in,
        })

    if _NC_CACHE is None:
        _NC_CACHE = _build_nc()
    res = run_bass_kernel_spmd(_NC_CACHE, in_maps, list(range(N_CORES)))
    LAST_RUN = res

    states = []
    for p in range(NUM_PART):
        rows = np.empty((NH, 2 * NH), np.float32)
        for j in range(SLICES):
            y8 = res.results[p * SLICES + j]["y"]  # [8, 128, 512]
            for m in range(2):
                for n in range(4):
                    rows[j * MROWS + m * 128:(j * MROWS) + (m + 1) * 128,
                         n * 512:(n + 1) * 512] = y8[m * 4 + n]
        states.append(rows.view(np.complex64).reshape(NH * NH))
    eig = np.float32(res.results[0]["eig"][0, 0])
    return (np.asarray(eig, np.float32), states[0], states[1])


# revision 23
# speedup vs baseline: 1.6276x; 1.0893x over previous
"""Distributed VQE kernel for 8 TRN2 NeuronCores.

Math: each part's circuit is
    |psi> = P . U2 . P . U1 |0>
where U1/U2 are tensor products of single-qubit RZ*RY gates and P is the
CNOT-ladder basis permutation |x> -> |prefix_xor(x)>.  U1|0> is a product
state; relabeling through P keeps a Markov (bond-2) structure, applying U2
keeps bond 2, and the final P relabeling lifts it to bond 4.  So the full
2^20-amplitude state is an MPS of bond dimension 4, i.e. the [1024, 1024]
amplitude matrix (high 10 bits x low 10 bits) factors exactly as
    state = A @ B,   A: [1024, 4] complex,  B: [4, 1024] complex.

Host work (tiny, O(2^10 * 16)): build A and B from the 160 params.
Device work (8 cores, SPMD, no collectives):
  - materialize the two states as K=8 fp32r matmuls producing interleaved
    (re, im) f32 pairs == complex64 memory layout, 2 MB written per core
    (cores 0-3: part 0 row-slices, cores 4-7: part 1 row-slices)
  - Pauli-Z expectation values via 4x4 Gram matrices:
        e_t = sum_ab Re( (A^T S_h conj(A))_ab * (B S_l B^H)_ab )
    computed with K=1024 PE matmuls over host-precomputed sign weights,
    then the coefficient-weighted eigenvalue (redundantly on every core).
"""

import numpy as np

import concourse.bacc as bacc
import concourse.mybir as mybir
from concourse.bass_utils import run_bass_kernel_spmd
from concourse.tile import TileContext

SUB_Q = 20
DEPTH = 2
HALF = SUB_Q // 2          # 10 qubits per A/B side
NH = 1 << HALF             # 1024
NUM_PART = 2
N_TERMS = 6
PARAMS_PER_PART = 2 * SUB_Q * DEPTH  # 80
N_CORES = 8
SLICES = 4                 # h-row slices per part (one per core)
MROWS = NH // SLICES       # 256 rows per core
F32 = mybir.dt.float32
F32R = mybir.dt.float32r   # single-pass PE fp32 (1 cyc/row at N>=256)

LAST_RUN = None  # BassKernelResults of the most recent device run (for test.py)
_NC_CACHE = None


# ---------------------------------------------------------------- host math
def _part_AB(theta):
    """theta [80] float64 -> A [1024, 4], B [4, 1024] complex128 with
    state[z] = A[z >> 10] @ B[:, z & 1023]."""
    th = np.asarray(theta, np.float64).reshape(DEPTH, 2, SUB_Q)
    a0, b0 = th[0, 0] * 0.5, th[0, 1] * 0.5
    u = np.stack([np.exp(-1j * b0) * np.cos(a0),
                  np.exp(1j * b0) * np.sin(a0)], axis=1)          # [20, 2]
    a1, b1 = th[1, 0] * 0.5, th[1, 1] * 0.5
    c, s = np.cos(a1), np.sin(a1)
    g = np.empty((SUB_Q, 2, 2), np.complex128)
    g[:, 0, 0] = np.exp(-1j * b1) * c
    g[:, 0, 1] = np.exp(-1j * b1) * (-s)
    g[:, 1, 0] = np.exp(1j * b1) * s
    g[:, 1, 1] = np.exp(1j * b1) * c

    # T_t(w)[al, be] = g[t, w, be] * u[t, be ^ al]
    # S_t(z)[(al, cb), (be, cb')] = T_t(z ^ cb)[al, be] * (cb' == z)
    S = np.zeros((SUB_Q, 2, 4, 4), np.complex128)
    for t in range(SUB_Q):
        for z in range(2):
            for al in range(2):
                for cb in range(2):
                    w = z ^ cb
                    for be in range(2):
                        S[t, z, al * 2 + cb, be * 2 + z] = g[t, w, be] * u[t, be ^ al]

    cur = np.zeros((1, 4), np.complex128)
    cur[0, 0] = 1.0                                   # (al=0, carried bit 0)
    for t in range(HALF):                             # bits z_0 (MSB) .. z_9
        n = cur.shape[0]
        nxt = np.empty((2 * n, 4), np.complex128)
        nxt[0::2] = cur @ S[t, 0]
        nxt[1::2] = cur @ S[t, 1]
        cur = nxt
    A = cur                                           # [1024, 4]

    cur = np.ones((4, 1), np.complex128)
    for k in range(HALF):                             # bits z_19 .. z_10
        t = SUB_Q - 1 - k
        n = cur.shape[1]
        nxt = np.empty((4, 2 * n), np.complex128)
        nxt[:, 0:n] = S[t, 0] @ cur
        nxt[:, n:2 * n] = S[t, 1] @ cur
        cur = nxt
    B = cur                                           # [4, 1024]
    return A, B


def _parity_signs(mask):
    """(-1)^popcount(i & mask) for i in [0, 1024), as float64 [1024]."""
    idx = np.arange(NH)
    r = idx & int(mask)
    p = np.zeros_like(idx)
    for b in range(HALF):
        p ^= (r >> b) & 1
    return (1.0 - 2.0 * p).astype(np.float64)


def _chunked(x):
    """[1024, F] -> [128, 8 * F] with h = chunk * 128 + partition."""
    f = x.shape[1]
    return np.ascontiguousarray(
        x.reshape(8, 128, f).transpose(1, 0, 2).reshape(128, 8 * f)
    ).astype(np.float32)


# ---------------------------------------------------------------- device IR
def _build_nc():
    nc = bacc.Bacc("TRN2", target_bir_lowering=False, debug=False,
                   enable_asserts=False)

    # lr: [lhsT | rhsB] packed -> one small critical-path DMA
    d_lr = nc.declare_dram_parameter("lr", [8, MROWS + 2 * NH], F32R,
                                     isOutput=False)
    d_apk = nc.declare_dram_parameter("apk", [128, 128], F32R, isOutput=False)
    d_wa = nc.declare_dram_parameter("wa", [128, 1024], F32R, isOutput=False)
    d_bpk = nc.declare_dram_parameter("bpk", [128, 128], F32R, isOutput=False)
    d_wb = nc.declare_dram_parameter("wb", [128, 1024], F32R, isOutput=False)
    # small: tmat [64, 6] ++ coefs (rows 0..5 of col 6)
    d_small = nc.declare_dram_parameter("small", [64, N_TERMS + 1], F32,
                                        isOutput=False)
    # output state slice, chunk-contiguous: [m*2+half, 128, 1024]
    d_y = nc.declare_dram_parameter("y", [4, 128, NH], F32, isOutput=True)
    d_eig = nc.declare_dram_parameter("eig", [1, 1], F32, isOutput=True)

    with TileContext(nc) as tc:
        with (
            tc.tile_pool(name="cpool", bufs=1) as cpool,
            tc.tile_pool(name="ypool", bufs=2) as ypool,
            tc.tile_pool(name="ypsum", bufs=3, space="PSUM") as ypsum_pool,
            tc.tile_pool(name="spsum", bufs=1, space="PSUM") as spsum_pool,
        ):
            # ---- load inputs
            sb_lr = cpool.tile([8, MROWS + 2 * NH], F32R)
            nc.sync.dma_start(sb_lr[:], d_lr[:])
            sb_lhsT = sb_lr[:, 0:MROWS]
            sb_rhsB = sb_lr[:, MROWS:MROWS + 2 * NH]
            sb_apk = cpool.tile([128, 128], F32R)
            nc.sync.dma_start(sb_apk[:], d_apk[:])
            sb_bpk = cpool.tile([128, 128], F32R)
            nc.sync.dma_start(sb_bpk[:], d_bpk[:])
            sb_wa = cpool.tile([128, 1024], F32R)
            nc.sync.dma_start(sb_wa[:, 0:512], d_wa[:, 0:512])
            nc.sync.dma_start(sb_wa[:, 512:1024], d_wa[:, 512:1024])
            sb_wb = cpool.tile([128, 1024], F32R)
            nc.sync.dma_start(sb_wb[:, 0:512], d_wb[:, 0:512])
            nc.sync.dma_start(sb_wb[:, 512:1024], d_wb[:, 512:1024])
            sb_small = cpool.tile([64, N_TERMS + 1], F32)
            nc.sync.dma_start(sb_small[:], d_small[:])
            sb_tmat = sb_small[:, 0:N_TERMS]
            sb_coefs = sb_small[0:N_TERMS, N_TERMS:N_TERMS + 1]

            # ---- state materialization: y[mm, 2l + c] = (A @ B) interleaved
            for m in range(MROWS // 128):
                y_sb = ypool.tile([128, 2 * NH], F32, tag="y_sb")
                for half in range(2):
                    yp = ypsum_pool.tile([128, NH], F32, tag="yp")
                    for n in range(2):
                        nc.tensor.matmul(
                            yp[:, n * 512:(n + 1) * 512],
                            sb_lhsT[:, m * 128:(m + 1) * 128],
                            sb_rhsB[:, half * NH + n * 512:
                                    half * NH + (n + 1) * 512],
                            start=True, stop=True,
                        )
                    dst = y_sb[:, half * NH:(half + 1) * NH]
                    if half == 0:
                        nc.vector.tensor_copy(dst, yp[:])
                    else:
                        nc.scalar.copy(dst, yp[:])
                    nc.sync.dma_start(d_y[m * 2 + half, :, :], dst)

            # ---- expectation values via Gram matrices
            # One PSUM bank [64, 64]; group g in cols g*16..g*16+16:
            #   g = 0: A side pl=0, 1: A pl=1, 2: B pl=0, 3: B pl=1
            # M_g[(q, a, t) partitions, (q', pl', be) free] = sum wa_pl * apk
            # partition index = q*32 + a*6 + t  (24..32 per q unused)
            m_ps = spsum_pool.tile([64, 64], F32)
            for g, (lhs, rhs) in enumerate((
                    (sb_wa, sb_apk), (sb_wa, sb_apk),
                    (sb_wb, sb_bpk), (sb_wb, sb_bpk))):
                pl = g % 2
                for c in range(8):
                    nc.tensor.matmul(
                        m_ps[:, g * 16:(g + 1) * 16],
                        lhs[:, c * 128 + pl * 64: c * 128 + pl * 64 + 64],
                        rhs[:, c * 16:(c + 1) * 16],
                        start=(c == 0), stop=(c == 7))
            ms = cpool.tile([64, 64], F32)
            nc.vector.tensor_copy(ms[:], m_ps[:])

            e_sb = []
            for q in range(NUM_PART):
                sl = slice(q * 32, q * 32 + 24)   # 32-aligned q block
                fr = q * 8                        # (q, pl'=0) free block
                fi = q * 8 + 4                    # (q, pl'=1) free block
                mare = cpool.tile([64, 4], F32, tag=f"mare{q}")
                nc.vector.tensor_add(mare[sl, :], ms[sl, fr:fr + 4],
                                     ms[sl, 16 + fi:16 + fi + 4])
                maim = cpool.tile([64, 4], F32, tag=f"maim{q}")
                nc.vector.tensor_sub(maim[sl, :], ms[sl, 16 + fr:16 + fr + 4],
                                     ms[sl, fi:fi + 4])
                mbre = cpool.tile([64, 4], F32, tag=f"mbre{q}")
                nc.vector.tensor_add(mbre[sl, :], ms[sl, 32 + fr:32 + fr + 4],
                                     ms[sl, 48 + fi:48 + fi + 4])
                mbim = cpool.tile([64, 4], F32, tag=f"mbim{q}")
                nc.vector.tensor_sub(mbim[sl, :], ms[sl, 48 + fr:48 + fr + 4],
                                     ms[sl, 32 + fi:32 + fi + 4])

                t1 = cpool.tile([64, 4], F32, tag=f"t1{q}")
                nc.vector.tensor_mul(t1[sl, :], mare[sl, :], mbre[sl, :])
                t2 = cpool.tile([64, 4], F32, tag=f"t2{q}")
                nc.vector.tensor_mul(t2[sl, :], maim[sl, :], mbim[sl, :])
                dd = cpool.tile([64, 4], F32, tag=f"dd{q}")
                nc.vector.tensor_sub(dd[sl, :], t1[sl, :], t2[sl, :])
                ddr = cpool.tile([64, 1], F32, tag=f"ddr{q}")
                nc.vector.reduce_sum(ddr[sl, :], dd[sl, :],
                                     axis=mybir.AxisListType.X)

                # e_q[t] = sum_a ddr[q*32 + a*6 + t]  via indicator matmul
                e_ps = spsum_pool.tile([N_TERMS, 1], F32, tag="esm")
                nc.tensor.matmul(e_ps[:], sb_tmat[sl, :], ddr[sl, :],
                                 start=True, stop=True)
                e_q = cpool.tile([N_TERMS, 1], F32, tag=f"e{q}")
                nc.vector.tensor_copy(e_q[:], e_ps[:])
                e_sb.append(e_q)

            ec = cpool.tile([N_TERMS, 1], F32)
            nc.vector.tensor_mul(ec[:], e_sb[0][:], e_sb[1][:])
            ec2 = cpool.tile([N_TERMS, 1], F32)
            nc.vector.tensor_mul(ec2[:], ec[:], sb_coefs)
            ones = cpool.tile([N_TERMS, 1], F32)
            nc.vector.memset(ones[:], 1.0)
            eig_ps = spsum_pool.tile([1, 1], F32, tag="esm")
            nc.tensor.matmul(eig_ps[:], ones[:], ec2[:], start=True, stop=True)
            eig_sb = cpool.tile([1, 1], F32)
            nc.vector.tensor_copy(eig_sb[:], eig_ps[:])
            nc.sync.dma_start(d_eig[:], eig_sb[:])

    nc.finalize()
    return nc


# ---------------------------------------------------------------- entry
def kernel(params, coefs, z_masks):
    global LAST_RUN, _NC_CACHE
    params = np.asarray(params, np.float64)
    coefs = np.asarray(coefs, np.float32)
    z_masks = np.asarray(z_masks)

    AB = [_part_AB(params[p * PARAMS_PER_PART:(p + 1) * PARAMS_PER_PART])
          for p in range(NUM_PART)]

    # shared (all-core) expectation inputs
    # apk free layout: a' = q*8 + pl'*4 + be
    # wa partition layout: m = pl*64 + q*32 + a*6 + t  (zeros at 24..32)
    apk = np.empty((NH, 16))
    wa = np.zeros((NH, 2, 2, 32))
    bpk = np.empty((NH, 16))
    wb = np.zeros((NH, 2, 2, 32))
    for q in range(NUM_PART):
        A, B = AB[q]
        Bt = B.T
        for pl, plane in enumerate((A.real, A.imag)):
            apk[:, q * 8 + pl * 4: q * 8 + pl * 4 + 4] = plane
        for pl, plane in enumerate((Bt.real, Bt.imag)):
            bpk[:, q * 8 + pl * 4: q * 8 + pl * 4 + 4] = plane
        sh = np.stack([_parity_signs(int(z_masks[q, t]) >> HALF)
                       for t in range(N_TERMS)])          # [6, 1024]
        sl = np.stack([_parity_signs(int(z_masks[q, t]) & (NH - 1))
                       for t in range(N_TERMS)])
        # [h, a, t] -> flattened a*6 + t within the 32-wide (pl, q) block
        wa[:, 0, q, :24] = (A.real[:, :, None] * sh.T[:, None, :]).reshape(NH, 24)
        wa[:, 1, q, :24] = (A.imag[:, :, None] * sh.T[:, None, :]).reshape(NH, 24)
        wb[:, 0, q, :24] = (Bt.real[:, :, None] * sl.T[:, None, :]).reshape(NH, 24)
        wb[:, 1, q, :24] = (Bt.imag[:, :, None] * sl.T[:, None, :]).reshape(NH, 24)

    apk_c = _chunked(apk)
    wa_c = _chunked(wa.reshape(NH, 128))
    bpk_c = _chunked(bpk)
    wb_c = _chunked(wb.reshape(NH, 128))
    small = np.zeros((64, N_TERMS + 1), np.float32)
    for q in range(2):
        for a in range(4):
            for t in range(N_TERMS):
                small[q * 32 + a * N_TERMS + t, t] = 1.0
    small[0:N_TERMS, N_TERMS] = coefs

    in_maps = []
    for core in range(N_CORES):
        p, j = divmod(core, SLICES)
        A, B = AB[p]
        h0 = j * MROWS
        lr = np.empty((8, MROWS + 2 * NH), np.float32)
        lr[0:4, 0:MROWS] = A.real[h0:h0 + MROWS].T
        lr[4:8, 0:MROWS] = A.imag[h0:h0 + MROWS].T
        rhsB = lr[:, MROWS:]
        rhsB[0:4, 0::2] = B.real
        rhsB[4:8, 0::2] = -B.imag
        rhsB[0:4, 1::2] = B.imag
        rhsB[4:8, 1::2] = B.real
        in_maps.append({
            "lr": lr, "apk": apk_c, "wa": wa_c,
            "bpk": bpk_c, "wb": wb_c, "small": small,
        })

    if _NC_CACHE is None:
        _NC_CACHE = _build_nc()
    res = run_bass_kernel_spmd(_NC_CACHE, in_maps, list(range(N_CORES)))
    LAST_RUN = res

    states = []
    for p in range(NUM_PART):
        rows = np.empty((NH, 2 * NH), np.float32)
        for j in range(SLICES):
            y4 = res.results[p * SLICES + j]["y"]  # [4, 128, 1024]
            for m in range(2):
                for half in range(2):
                    rows[j * MROWS + m * 128:j * MROWS + (m + 1) * 128,
                         half * NH:(half + 1) * NH] = y4[m * 2 + half]
        states.append(rows.view(np.complex64).reshape(NH * NH))
    eig = np.float32(res.results[0]["eig"][0, 0])
    return (np.asarray(eig, np.float32), states[0], states[1])
